# revision 9
# baseline (speedup 1.0000x reference)
"""Trainium2 Bass kernel for CrossFrameSimilarityRefiner.

Computation (per batch element b, fully batch-parallel -> B=8 sharded over 8 cores):
  f = features[:, b]                      # [T, C, P]  T=16, C=256, P=1024
  ss[t,p] = sum_c f^2 ; sm[t,p] = sum_c f ; gm[t,p] = sum_c (f>0)
  S[t,p]  = sm / sqrt(ss)                 # == sum/||.|| (eps clamp irrelevant for randn)
  M'[s,p] = gm  (affine transform of mean(sign(f)) -> identical per-row ranking)
  scores[t,s] = sum_p S[t,p] * M'[s,p]    # row-wise ranking == reference ranking
  mask diag, top-3 indices s* ; compressed c* = s* - (s* > t)   (reference's faithful bug:
  c* indexes the ORIGINAL frame axis)
  out[t] = (W/3) @ (f[c*0]+f[c*1]+f[c*2]) + b

Layout: features kept in SBUF c-major ([c_chunk partitions, t, p] free);
column reductions done on the PE via one-hot column-selector matmuls so each
t lands on its own PSUM partition; the 3-frame gather uses register-indexed
dynamic SBUF slices (bf16) and the final matmul runs in fp32.
"""

import numpy as np

import concourse.bacc as bacc
import concourse.bass as bass
import concourse.tile as tile
from concourse import mybir
from concourse.bass_utils import run_bass_kernel_spmd

FP32 = mybir.dt.float32
F16 = mybir.dt.float16
I32 = mybir.dt.int32
U32 = mybir.dt.uint32
AF = mybir.ActivationFunctionType
OP = mybir.AluOpType

N_CORES = 8
BIG = 1.0e30


def _emit(nc, tc, T, C, P, K, handles):
    feat_h = handles["features"]
    out_h = handles["out"]
    sdbg_h = handles["scores_dbg"]
    idbg_h = handles["idx_dbg"]
    CC = C // 128          # c chunks (2)
    PH = P // 512          # psum-width chunks of p (2)
    PB = P // 128          # 128-blocks of p (8)
    DC = C // 128          # d chunks for output (2)

    with tc.tile_pool(name="persist", bufs=1) as pp:
        # ---- constants (from DRAM inputs) ----
        wt3_sb = pp.tile([128, CC, C], FP32, tag="wt3")
        bcol_sb = pp.tile([128, DC], FP32, tag="bcol")
        esel_sb = pp.tile([128, T * T], FP32, tag="esel")
        i16_sb = pp.tile([T, T], FP32, tag="i16")
        diag_sb = pp.tile([T, T], FP32, tag="diag")
        tcol_sb = pp.tile([T, 1], FP32, tag="tcol")
        for name, t_ in (("wt3", wt3_sb), ("bcol", bcol_sb), ("esel", esel_sb),
                         ("i16", i16_sb), ("diagbig", diag_sb), ("tcol", tcol_sb)):
            nc.sync.dma_start(t_[:], handles[name].ap())

        # ---- persistent state ----
        f16_sb = pp.tile([128, CC, T * P], F16, tag="f16")
        sm_sb = pp.tile([T, P], FP32, tag="sm")
        ss_sb = pp.tile([T, P], FP32, tag="ss")
        gm_sb = pp.tile([T, P], FP32, tag="gm")
        rn_sb = pp.tile([T, P], FP32, tag="rn")
        rs_sb = pp.tile([T, P], FP32, tag="rs")
        s_sb = pp.tile([T, P], FP32, tag="S")
        spt_sb = pp.tile([128, PB, T], FP32, tag="SpT")
        mpt_sb = pp.tile([128, PB, T], FP32, tag="MpT")
        scores_sb = pp.tile([T, T], FP32, tag="scores")
        maxv_sb = pp.tile([T, 8], FP32, tag="maxv")
        maxi_sb = pp.tile([T, 8], U32, tag="maxi")
        idxf_sb = pp.tile([T, K], FP32, tag="idxf")
        gt_sb = pp.tile([T, K], FP32, tag="gt")
        cidxf_sb = pp.tile([T, K], FP32, tag="cidxf")
        cidx_sb = pp.tile([T, K], I32, tag="cidx")
        row_sb = pp.tile([1, T * K], I32, tag="row")

        # ================= Phase A: stream in, stats =================
        with tc.tile_pool(name="statsps", bufs=1, space="PSUM") as sps, \
             tc.tile_pool(name="stream", bufs=4) as sp:
            sm_ps = [sps.tile([T, 512], FP32, tag=f"smp{ph}", name=f"smp{ph}")
                     for ph in range(PH)]
            ss_ps = [sps.tile([T, 512], FP32, tag=f"ssp{ph}", name=f"ssp{ph}")
                     for ph in range(PH)]
            gm_ps = [sps.tile([T, 512], FP32, tag=f"gmp{ph}", name=f"gmp{ph}")
                     for ph in range(PH)]

            for t in range(T):
                for cc in range(CC):
                    fch = sp.tile([128, P], FP32, tag="fch")
                    nc.sync.dma_start(fch[:], feat_h[t, cc * 128:(cc + 1) * 128, :])
                    # bf16 copy for the gather/combine phase
                    nc.vector.tensor_copy(f16_sb[:, cc, t * P:(t + 1) * P], fch[:])
                    sq = sp.tile([128, P], FP32, tag="sq")
                    nc.scalar.activation(sq[:], fch[:], AF.Square)
                    gsc = sp.tile([128, P], FP32, tag="gsc")
                    nc.vector.tensor_scalar(gsc[:], fch[:], 0.0, None, OP.is_gt)
                    st = (t == 0 and cc == 0)
                    sx = (t == T - 1 and cc == CC - 1)
                    lhs = esel_sb[:, T * t:T * (t + 1)]
                    for ph in range(PH):
                        sl = slice(ph * 512, (ph + 1) * 512)
                        nc.tensor.matmul(sm_ps[ph][:], lhs, fch[:, sl], start=st, stop=sx)
                        nc.tensor.matmul(ss_ps[ph][:], lhs, sq[:, sl], start=st, stop=sx)
                        nc.tensor.matmul(gm_ps[ph][:], lhs, gsc[:, sl], start=st, stop=sx)

            for ph in range(PH):
                sl = slice(ph * 512, (ph + 1) * 512)
                nc.scalar.copy(sm_sb[:, sl], sm_ps[ph][:])
                nc.scalar.copy(ss_sb[:, sl], ss_ps[ph][:])
                nc.scalar.copy(gm_sb[:, sl], gm_ps[ph][:])

        # ================= Phase B: scores + top-k =================
        with tc.tile_pool(name="bps", bufs=2, space="PSUM") as bps:
            nc.scalar.activation(rn_sb[:], ss_sb[:], AF.Sqrt)
            nc.vector.reciprocal(rs_sb[:], rn_sb[:])
            nc.vector.tensor_mul(s_sb[:], sm_sb[:], rs_sb[:])

            for src, dst in ((s_sb, spt_sb), (gm_sb, mpt_sb)):
                for pb in range(PB):
                    tr = bps.tile([128, T], FP32, tag="tr")
                    nc.tensor.transpose(tr[:], src[:, pb * 128:(pb + 1) * 128], i16_sb[:])
                    nc.scalar.copy(dst[:, pb, :], tr[:])

            sc_ps = bps.tile([T, T], FP32, tag="scps")
            for pb in range(PB):
                nc.tensor.matmul(sc_ps[:], spt_sb[:, pb, :], mpt_sb[:, pb, :],
                                 start=(pb == 0), stop=(pb == PB - 1))
            # exclude s == t, move to SBUF
            nc.vector.tensor_sub(scores_sb[:], sc_ps[:], diag_sb[:])

            nc.vector.max(maxv_sb[:], scores_sb[:])
            nc.vector.max_index(maxi_sb[:], maxv_sb[:], scores_sb[:])
            # compressed index c* = s* - (s* > t)   (faithful reference bug)
            nc.vector.tensor_copy(idxf_sb[:], maxi_sb[:, 0:K])
            nc.vector.tensor_scalar(gt_sb[:], idxf_sb[:], tcol_sb[:, 0:1], None, OP.is_gt)
            nc.vector.tensor_sub(cidxf_sb[:], idxf_sb[:], gt_sb[:])
            nc.vector.tensor_copy(cidx_sb[:], cidxf_sb[:])
            nc.sync.dma_start(row_sb[:], cidx_sb[:])
            # debug outputs
            nc.sync.dma_start(sdbg_h.ap(), scores_sb[:])
            nc.sync.dma_start(idbg_h.ap(), row_sb[:])

        # ================= Phase C: gather-combine + linear =================
        with tc.tile_pool(name="cps", bufs=4, space="PSUM") as cps, \
             tc.tile_pool(name="cpool", bufs=2) as cp:
            for t in range(T):
                _, vals = nc.values_load_multi_w_load_instructions(
                    row_sb[0:1, K * t:K * (t + 1)],
                    engines=bass.OrderedSet([mybir.EngineType.DVE]),
                    min_val=0, max_val=T - 2,
                    skip_runtime_bounds_check=True,
                )
                mf16 = cp.tile([128, CC, P], F16, tag="mf16")
                for cc in range(CC):
                    a0 = f16_sb[:, cc, bass.ds(vals[0] * P, P)]
                    a1 = f16_sb[:, cc, bass.ds(vals[1] * P, P)]
                    nc.vector.tensor_add(mf16[:, cc, :], a0, a1)
                    for k in range(2, K):
                        ak = f16_sb[:, cc, bass.ds(vals[k] * P, P)]
                        nc.vector.tensor_add(mf16[:, cc, :], mf16[:, cc, :], ak)
                mf32 = cp.tile([128, CC, P], FP32, tag="mf32")
                nc.vector.tensor_copy(mf32[:], mf16[:])

                for dc in range(DC):
                    osb = cp.tile([128, P], FP32, tag="osb")
                    for ph in range(PH):
                        po = cps.tile([128, 512], FP32, tag="po")
                        for cc in range(CC):
                            nc.tensor.matmul(
                                po[:],
                                wt3_sb[:, cc, dc * 128:(dc + 1) * 128],
                                mf32[:, cc, ph * 512:(ph + 1) * 512],
                                start=(cc == 0), stop=(cc == CC - 1),
                            )
                        nc.scalar.activation(osb[:, ph * 512:(ph + 1) * 512], po[:],
                                             AF.Identity, bias=bcol_sb[:, dc:dc + 1])
                    nc.sync.dma_start(out_h[t, dc * 128:(dc + 1) * 128, :], osb[:])


def build_program(T=16, C=256, P=1024, K=3):
    nc = bacc.Bacc("TRN2", target_bir_lowering=False, debug=False,
                   num_devices=N_CORES)
    handles = {}
    handles["features"] = nc.dram_tensor("features", [T, C, P], FP32,
                                         kind="ExternalInput")
    for name, shape, dt in (
        ("wt3", [128, C // 128, C], FP32),
        ("bcol", [128, C // 128], FP32),
        ("esel", [128, T * T], FP32),
        ("i16", [T, T], FP32),
        ("diagbig", [T, T], FP32),
        ("tcol", [T, 1], FP32),
    ):
        handles[name] = nc.dram_tensor(name, shape, dt, kind="ExternalInput")
    handles["out"] = nc.dram_tensor("out", [T, C, P], FP32, kind="ExternalOutput")
    handles["scores_dbg"] = nc.dram_tensor("scores_dbg", [T, T], FP32,
                                           kind="ExternalOutput")
    handles["idx_dbg"] = nc.dram_tensor("idx_dbg", [1, T * K], I32,
                                        kind="ExternalOutput")

    with tile.TileContext(nc) as tc:
        _emit(nc, tc, T, C, P, K, handles)
    nc.compile()
    return nc


def _host_consts(W, b, T, C, K):
    consts = {}
    wt3 = (np.asarray(W, np.float32).T / float(K)).astype(np.float32)  # [C, C] (c, d)
    # SBUF tile is [c_in(partition), cc, d]
    consts["wt3"] = np.ascontiguousarray(
        wt3.reshape(C // 128, 128, C).transpose(1, 0, 2))
    consts["bcol"] = np.ascontiguousarray(
        np.asarray(b, np.float32).reshape(C // 128, 128).T)
    esel = np.zeros((128, T * T), np.float32)
    for t in range(T):
        esel[:, T * t + t] = 1.0
    consts["esel"] = esel
    consts["i16"] = np.eye(T, dtype=np.float32)
    consts["diagbig"] = (np.eye(T, dtype=np.float32) * BIG).astype(np.float32)
    consts["tcol"] = np.arange(T, dtype=np.float32).reshape(T, 1)
    return consts


_CACHE = {}


def kernel(features, W, b, top_k):
    features = np.asarray(features, np.float32)
    T, B, C, H, Wd = features.shape
    P = H * Wd
    K = int(top_k)
    assert B == N_CORES and C == 256 and P == 1024 and T == 16 and K == 3

    key = (T, C, P, K)
    if key not in _CACHE:
        _CACHE[key] = build_program(T, C, P, K)
    nc = _CACHE[key]

    consts = _host_consts(W, b, T, C, K)
    feat = features.reshape(T, B, C, P)
    in_maps = [
        {"features": np.ascontiguousarray(feat[:, i]), **consts}
        for i in range(N_CORES)
    ]
    res = run_bass_kernel_spmd(nc, in_maps, list(range(N_CORES)))
    out = np.stack([res.results[i]["out"] for i in range(N_CORES)], axis=1)
    return np.ascontiguousarray(out.reshape(T, B, C, H, Wd))


# revision 18
# speedup vs baseline: 1.7510x; 1.7510x over previous
"""Trainium2 Bass kernel for CrossFrameSimilarityRefiner.

Computation (per batch element b, fully batch-parallel -> B=8 sharded over 8 cores):
  f = features[:, b]                      # [T, C, P]  T=16, C=256, P=1024
  ss[t,p] = sum_c f^2 ; sm[t,p] = sum_c f ; gm[t,p] = sum_c (f>0)
  S[t,p]  = sm / sqrt(ss)                 # == sum/||.|| (eps clamp irrelevant for randn)
  M'[s,p] = gm  (affine transform of mean(sign(f)) -> identical per-row ranking)
  scores[t,s] = sum_p S[t,p] * M'[s,p]    # row-wise ranking == reference ranking
  mask diag, top-3 indices s* ; compressed c* = s* - (s* > t)   (reference's faithful bug:
  c* indexes the ORIGINAL frame axis)
  out[t] = (W/3) @ (f[c*0]+f[c*1]+f[c*2]) + b

Layout: features kept in SBUF c-major ([c_chunk partitions, t, p] free);
column reductions done on the PE via one-hot column-selector matmuls so each
t lands on its own PSUM partition; the 3-frame gather uses register-indexed
dynamic SBUF slices (bf16) and the final matmul runs in fp32.
"""

import numpy as np

import concourse.bacc as bacc
import concourse.bass as bass
import concourse.tile as tile
from concourse import mybir
from concourse.bass_utils import run_bass_kernel_spmd

FP32 = mybir.dt.float32
F32R = mybir.dt.float32r
F16 = mybir.dt.float16
I32 = mybir.dt.int32
U32 = mybir.dt.uint32
AF = mybir.ActivationFunctionType
OP = mybir.AluOpType

N_CORES = 8
BIG = 1.0e30


def _emit(nc, tc, T, C, P, K, handles):
    feat_h = handles["features"]
    out_h = handles["out"]
    sdbg_h = handles["scores_dbg"]
    idbg_h = handles["idx_dbg"]
    CC = C // 128          # c chunks (2)
    PH = P // 512          # psum-width chunks of p (2)
    PB = P // 128          # 128-blocks of p (8)
    DC = C // 128          # d chunks for output (2)

    with tc.tile_pool(name="persist", bufs=1) as pp:
        # ---- constants (from DRAM inputs) ----
        wt3_sb = pp.tile([128, CC, C], F32R, tag="wt3")
        bcol_sb = pp.tile([128, DC], FP32, tag="bcol")
        esel_sb = pp.tile([128, T * T], F16, tag="esel")
        i16_sb = pp.tile([T, T], FP32, tag="i16")
        diag_sb = pp.tile([T, T], FP32, tag="diag")
        tcol_sb = pp.tile([T, 1], FP32, tag="tcol")
        for name, t_ in (("wt3", wt3_sb), ("bcol", bcol_sb), ("esel", esel_sb),
                         ("i16", i16_sb), ("diagbig", diag_sb), ("tcol", tcol_sb)):
            nc.sync.dma_start(t_[:], handles[name].ap())

        # ---- persistent state ----
        f16_sb = pp.tile([128, CC, T * P], F16, tag="f16")
        sm_sb = pp.tile([T, P], FP32, tag="sm")
        ss_sb = pp.tile([T, P], FP32, tag="ss")
        gm_sb = pp.tile([T, P], FP32, tag="gm")
        rn_sb = pp.tile([T, P], FP32, tag="rn")
        rs_sb = pp.tile([T, P], FP32, tag="rs")
        s_sb = pp.tile([T, P], FP32, tag="S")
        spt_sb = pp.tile([128, PB, T], FP32, tag="SpT")
        mpt_sb = pp.tile([128, PB, T], FP32, tag="MpT")
        scores_sb = pp.tile([T, T], FP32, tag="scores")
        maxv_sb = pp.tile([T, 8], FP32, tag="maxv")
        maxi_sb = pp.tile([T, 8], U32, tag="maxi")
        idxf_sb = pp.tile([T, K], FP32, tag="idxf")
        gt_sb = pp.tile([T, K], FP32, tag="gt")
        cidxf_sb = pp.tile([T, K], FP32, tag="cidxf")
        cidx_sb = pp.tile([T, K], I32, tag="cidx")
        row_sb = pp.tile([1, T * K], I32, tag="row")

        # ================= Phase A: stream in, stats =================
        with tc.tile_pool(name="statsps", bufs=1, space="PSUM") as sps, \
             tc.tile_pool(name="stream", bufs=4) as sp:
            sm_ps = [sps.tile([T, 512], FP32, tag=f"smp{ph}", name=f"smp{ph}")
                     for ph in range(PH)]
            ss_ps = [sps.tile([T, 512], FP32, tag=f"ssp{ph}", name=f"ssp{ph}")
                     for ph in range(PH)]
            gm_ps = [sps.tile([T, 512], FP32, tag=f"gmp{ph}", name=f"gmp{ph}")
                     for ph in range(PH)]

            for t in range(T):
                for cc in range(CC):
                    fch = sp.tile([128, P], FP32, tag="fch")
                    nc.sync.dma_start(fch[:], feat_h[t, cc * 128:(cc + 1) * 128, :])
                    # fp16 copy: used by the stats matmuls AND the gather phase
                    f16c = f16_sb[:, cc, t * P:(t + 1) * P]
                    nc.vector.tensor_copy(f16c, fch[:])
                    sq = sp.tile([128, P], F16, tag="sq")
                    nc.scalar.activation(sq[:], fch[:], AF.Square)
                    gsc = sp.tile([128, P], F16, tag="gsc")
                    nc.vector.tensor_scalar(gsc[:], fch[:], 0.0, None, OP.is_gt)
                    st = (t == 0 and cc == 0)
                    sx = (t == T - 1 and cc == CC - 1)
                    lhs = esel_sb[:, T * t:T * (t + 1)]
                    for ph in range(PH):
                        sl = slice(ph * 512, (ph + 1) * 512)
                        nc.tensor.matmul(sm_ps[ph][:], lhs, f16c[:, sl], start=st, stop=sx)
                        nc.tensor.matmul(ss_ps[ph][:], lhs, sq[:, sl], start=st, stop=sx)
                        nc.tensor.matmul(gm_ps[ph][:], lhs, gsc[:, sl], start=st, stop=sx)

            for ph in range(PH):
                sl = slice(ph * 512, (ph + 1) * 512)
                nc.scalar.copy(sm_sb[:, sl], sm_ps[ph][:])
                nc.scalar.copy(ss_sb[:, sl], ss_ps[ph][:])
                nc.scalar.copy(gm_sb[:, sl], gm_ps[ph][:])

        # ================= Phase B: scores + top-k =================
        with tc.tile_pool(name="bps", bufs=2, space="PSUM") as bps:
            nc.scalar.activation(rn_sb[:], ss_sb[:], AF.Sqrt)
            nc.vector.reciprocal(rs_sb[:], rn_sb[:])
            nc.vector.tensor_mul(s_sb[:], sm_sb[:], rs_sb[:])

            for src, dst in ((s_sb, spt_sb), (gm_sb, mpt_sb)):
                for pb in range(PB):
                    tr = bps.tile([128, T], FP32, tag="tr")
                    nc.tensor.transpose(tr[:], src[:, pb * 128:(pb + 1) * 128], i16_sb[:])
                    nc.scalar.copy(dst[:, pb, :], tr[:])

            sc_ps = bps.tile([T, T], FP32, tag="scps")
            for pb in range(PB):
                nc.tensor.matmul(sc_ps[:], spt_sb[:, pb, :], mpt_sb[:, pb, :],
                                 start=(pb == 0), stop=(pb == PB - 1))
            # exclude s == t, move to SBUF
            nc.vector.tensor_sub(scores_sb[:], sc_ps[:], diag_sb[:])

            nc.vector.max(maxv_sb[:], scores_sb[:])
            nc.vector.max_index(maxi_sb[:], maxv_sb[:], scores_sb[:])
            # compressed index c* = s* - (s* > t)   (faithful reference bug)
            nc.vector.tensor_copy(idxf_sb[:], maxi_sb[:, 0:K])
            nc.vector.tensor_scalar(gt_sb[:], idxf_sb[:], tcol_sb[:, 0:1], None, OP.is_gt)
            nc.vector.tensor_sub(cidxf_sb[:], idxf_sb[:], gt_sb[:])
            nc.vector.tensor_copy(cidx_sb[:], cidxf_sb[:])
            nc.sync.dma_start(row_sb[:], cidx_sb[:])
            # debug outputs
            nc.sync.dma_start(sdbg_h.ap(), scores_sb[:])
            nc.sync.dma_start(idbg_h.ap(), row_sb[:])

        # ================= Phase C: gather-combine + linear =================
        with tc.tile_pool(name="cps", bufs=4, space="PSUM") as cps, \
             tc.tile_pool(name="cpool", bufs=2) as cp:
            for t in range(T):
                _, vals = nc.values_load_multi_w_load_instructions(
                    row_sb[0:1, K * t:K * (t + 1)],
                    engines=bass.OrderedSet([mybir.EngineType.DVE]),
                    min_val=0, max_val=T - 2,
                    skip_runtime_bounds_check=True,
                )
                mf16 = cp.tile([128, CC, P], F16, tag="mf16")
                for cc in range(CC):
                    a0 = f16_sb[:, cc, bass.ds(vals[0] * P, P)]
                    a1 = f16_sb[:, cc, bass.ds(vals[1] * P, P)]
                    nc.vector.tensor_add(mf16[:, cc, :], a0, a1)
                    for k in range(2, K):
                        ak = f16_sb[:, cc, bass.ds(vals[k] * P, P)]
                        nc.vector.tensor_add(mf16[:, cc, :], mf16[:, cc, :], ak)
                mf32 = cp.tile([128, CC, P], F32R, tag="mf32")
                nc.vector.tensor_copy(mf32[:], mf16[:])

                for dc in range(DC):
                    osb = cp.tile([128, P], FP32, tag="osb")
                    for ph in range(PH):
                        po = cps.tile([128, 512], FP32, tag="po")
                        for cc in range(CC):
                            # float32r: full-rate PE at ~fp32 precision
                            nc.tensor.matmul(
                                po[:],
                                wt3_sb[:, cc, dc * 128:(dc + 1) * 128],
                                mf32[:, cc, ph * 512:(ph + 1) * 512],
                                start=(cc == 0), stop=(cc == CC - 1),
                            )
                        nc.scalar.activation(osb[:, ph * 512:(ph + 1) * 512], po[:],
                                             AF.Identity, bias=bcol_sb[:, dc:dc + 1])
                    nc.sync.dma_start(out_h[t, dc * 128:(dc + 1) * 128, :], osb[:])


def build_program(T=16, C=256, P=1024, K=3):
    nc = bacc.Bacc("TRN2", target_bir_lowering=False, debug=False,
                   num_devices=N_CORES)
    handles = {}
    handles["features"] = nc.dram_tensor("features", [T, C, P], FP32,
                                         kind="ExternalInput")
    for name, shape, dt in (
        ("wt3", [128, C // 128, C], F32R),
        ("bcol", [128, C // 128], FP32),
        ("esel", [128, T * T], F16),
        ("i16", [T, T], FP32),
        ("diagbig", [T, T], FP32),
        ("tcol", [T, 1], FP32),
    ):
        handles[name] = nc.dram_tensor(name, shape, dt, kind="ExternalInput")
    handles["out"] = nc.dram_tensor("out", [T, C, P], FP32, kind="ExternalOutput")
    handles["scores_dbg"] = nc.dram_tensor("scores_dbg", [T, T], FP32,
                                           kind="ExternalOutput")
    handles["idx_dbg"] = nc.dram_tensor("idx_dbg", [1, T * K], I32,
                                        kind="ExternalOutput")

    with tile.TileContext(nc) as tc:
        _emit(nc, tc, T, C, P, K, handles)
    nc.compile()
    return nc


def _host_consts(W, b, T, C, K):
    consts = {}
    wt3 = (np.asarray(W, np.float32).T / float(K)).astype(np.float32)  # [C, C] (c, d)
    # SBUF tile is [c_in(partition), cc, d]
    consts["wt3"] = np.ascontiguousarray(
        wt3.reshape(C // 128, 128, C).transpose(1, 0, 2))
    consts["bcol"] = np.ascontiguousarray(
        np.asarray(b, np.float32).reshape(C // 128, 128).T)
    esel = np.zeros((128, T * T), np.float16)
    for t in range(T):
        esel[:, T * t + t] = 1.0
    consts["esel"] = esel
    consts["i16"] = np.eye(T, dtype=np.float32)
    consts["diagbig"] = (np.eye(T, dtype=np.float32) * BIG).astype(np.float32)
    consts["tcol"] = np.arange(T, dtype=np.float32).reshape(T, 1)
    return consts


_CACHE = {}


def kernel(features, W, b, top_k):
    features = np.asarray(features, np.float32)
    T, B, C, H, Wd = features.shape
    P = H * Wd
    K = int(top_k)
    assert B == N_CORES and C == 256 and P == 1024 and T == 16 and K == 3

    key = (T, C, P, K)
    if key not in _CACHE:
        _CACHE[key] = build_program(T, C, P, K)
    nc = _CACHE[key]

    consts = _host_consts(W, b, T, C, K)
    feat = features.reshape(T, B, C, P)
    in_maps = [
        {"features": np.ascontiguousarray(feat[:, i]), **consts}
        for i in range(N_CORES)
    ]
    res = run_bass_kernel_spmd(nc, in_maps, list(range(N_CORES)))
    out = np.stack([res.results[i]["out"] for i in range(N_CORES)], axis=1)
    return np.ascontiguousarray(out.reshape(T, B, C, H, Wd))


# revision 20
# speedup vs baseline: 1.8248x; 1.0421x over previous
"""Trainium2 Bass kernel for CrossFrameSimilarityRefiner.

Computation (per batch element b, fully batch-parallel -> B=8 sharded over 8 cores):
  f = features[:, b]                      # [T, C, P]  T=16, C=256, P=1024
  ss[t,p] = sum_c f^2 ; sm[t,p] = sum_c f ; gm[t,p] = sum_c (f>0)
  S[t,p]  = sm / sqrt(ss)                 # == sum/||.|| (eps clamp irrelevant for randn)
  M'[s,p] = gm  (affine transform of mean(sign(f)) -> identical per-row ranking)
  scores[t,s] = sum_p S[t,p] * M'[s,p]    # row-wise ranking == reference ranking
  mask diag, top-3 indices s* ; compressed c* = s* - (s* > t)   (reference's faithful bug:
  c* indexes the ORIGINAL frame axis)
  out[t] = (W/3) @ (f[c*0]+f[c*1]+f[c*2]) + b

Layout: features kept in SBUF c-major ([c_chunk partitions, t, p] free);
column reductions done on the PE via one-hot column-selector matmuls so each
t lands on its own PSUM partition; the 3-frame gather uses register-indexed
dynamic SBUF slices (bf16) and the final matmul runs in fp32.
"""

import numpy as np

import concourse.bacc as bacc
import concourse.bass as bass
import concourse.tile as tile
from concourse import mybir
from concourse.bass_utils import run_bass_kernel_spmd

FP32 = mybir.dt.float32
F32R = mybir.dt.float32r
F16 = mybir.dt.float16
I32 = mybir.dt.int32
U32 = mybir.dt.uint32
AF = mybir.ActivationFunctionType
OP = mybir.AluOpType

N_CORES = 8
BIG = 1.0e30


def _emit(nc, tc, T, C, P, K, handles):
    feat_h = handles["features"]
    out_h = handles["out"]
    sdbg_h = handles["scores_dbg"]
    idbg_h = handles["idx_dbg"]
    CC = C // 128          # c chunks (2)
    PH = P // 512          # psum-width chunks of p (2)
    PB = P // 128          # 128-blocks of p (8)
    DC = C // 128          # d chunks for output (2)

    with tc.tile_pool(name="persist", bufs=1) as pp:
        # ---- constants (from DRAM inputs) ----
        wt3_sb = pp.tile([128, CC, C], F32R, tag="wt3")
        bcol_sb = pp.tile([128, DC], FP32, tag="bcol")
        esel_sb = pp.tile([128, T * T], F16, tag="esel")
        i16_sb = pp.tile([T, T], FP32, tag="i16")
        diag_sb = pp.tile([T, T], FP32, tag="diag")
        tcol_sb = pp.tile([T, 1], FP32, tag="tcol")
        for name, t_ in (("wt3", wt3_sb), ("bcol", bcol_sb), ("esel", esel_sb),
                         ("i16", i16_sb), ("diagbig", diag_sb), ("tcol", tcol_sb)):
            nc.sync.dma_start(t_[:], handles[name].ap())

        # ---- persistent state ----
        f16_sb = pp.tile([128, CC, T * P], F16, tag="f16")
        sm_sb = pp.tile([T, P], FP32, tag="sm")
        ss_sb = pp.tile([T, P], FP32, tag="ss")
        gm_sb = pp.tile([T, P], FP32, tag="gm")
        rn_sb = pp.tile([T, P], FP32, tag="rn")
        rs_sb = pp.tile([T, P], FP32, tag="rs")
        s_sb = pp.tile([T, P], FP32, tag="S")
        spt_sb = pp.tile([128, PB, T], FP32, tag="SpT")
        mpt_sb = pp.tile([128, PB, T], FP32, tag="MpT")
        scores_sb = pp.tile([T, T], FP32, tag="scores")
        maxv_sb = pp.tile([T, 8], FP32, tag="maxv")
        maxi_sb = pp.tile([T, 8], U32, tag="maxi")
        idxf_sb = pp.tile([T, K], FP32, tag="idxf")
        gt_sb = pp.tile([T, K], FP32, tag="gt")
        cidxf_sb = pp.tile([T, K], FP32, tag="cidxf")
        cidx_sb = pp.tile([T, K], I32, tag="cidx")
        row_sb = pp.tile([1, T * K], I32, tag="row")

        # ================= Phase A: stream in, stats =================
        with tc.tile_pool(name="statsps", bufs=1, space="PSUM") as sps, \
             tc.tile_pool(name="stream", bufs=4) as sp:
            sm_ps = [sps.tile([T, 512], FP32, tag=f"smp{ph}", name=f"smp{ph}")
                     for ph in range(PH)]
            ss_ps = [sps.tile([T, 512], FP32, tag=f"ssp{ph}", name=f"ssp{ph}")
                     for ph in range(PH)]
            gm_ps = [sps.tile([T, 512], FP32, tag=f"gmp{ph}", name=f"gmp{ph}")
                     for ph in range(PH)]

            for t in range(T):
                for cc in range(CC):
                    fch = sp.tile([128, P], FP32, tag="fch")
                    nc.sync.dma_start(fch[:], feat_h[t, cc * 128:(cc + 1) * 128, :])
                    # fp16 copy: used by the stats matmuls AND the gather phase
                    f16c = f16_sb[:, cc, t * P:(t + 1) * P]
                    nc.vector.tensor_copy(f16c, fch[:])
                    sq = sp.tile([128, P], F16, tag="sq")
                    nc.scalar.activation(sq[:], fch[:], AF.Square)
                    gsc = sp.tile([128, P], F16, tag="gsc")
                    nc.vector.tensor_scalar(gsc[:], fch[:], 0.0, None, OP.is_gt)
                    st = (t == 0 and cc == 0)
                    sx = (t == T - 1 and cc == CC - 1)
                    lhs = esel_sb[:, T * t:T * (t + 1)]
                    for ph in range(PH):
                        sl = slice(ph * 512, (ph + 1) * 512)
                        nc.tensor.matmul(sm_ps[ph][:], lhs, f16c[:, sl], start=st, stop=sx)
                        nc.tensor.matmul(ss_ps[ph][:], lhs, sq[:, sl], start=st, stop=sx)
                        nc.tensor.matmul(gm_ps[ph][:], lhs, gsc[:, sl], start=st, stop=sx)

            for ph in range(PH):
                sl = slice(ph * 512, (ph + 1) * 512)
                nc.scalar.copy(sm_sb[:, sl], sm_ps[ph][:])
                nc.scalar.copy(ss_sb[:, sl], ss_ps[ph][:])
                nc.scalar.copy(gm_sb[:, sl], gm_ps[ph][:])

        # ================= Phase B: scores + top-k =================
        with tc.tile_pool(name="bps", bufs=2, space="PSUM") as bps:
            nc.scalar.activation(rn_sb[:], ss_sb[:], AF.Sqrt)
            nc.vector.reciprocal(rs_sb[:], rn_sb[:])
            nc.vector.tensor_mul(s_sb[:], sm_sb[:], rs_sb[:])

            for src, dst in ((s_sb, spt_sb), (gm_sb, mpt_sb)):
                for pb in range(PB):
                    tr = bps.tile([128, T], FP32, tag="tr")
                    nc.tensor.transpose(tr[:], src[:, pb * 128:(pb + 1) * 128], i16_sb[:])
                    nc.scalar.copy(dst[:, pb, :], tr[:])

            sc_ps = bps.tile([T, T], FP32, tag="scps")
            for pb in range(PB):
                nc.tensor.matmul(sc_ps[:], spt_sb[:, pb, :], mpt_sb[:, pb, :],
                                 start=(pb == 0), stop=(pb == PB - 1))
            # exclude s == t, move to SBUF
            nc.vector.tensor_sub(scores_sb[:], sc_ps[:], diag_sb[:])

            nc.vector.max(maxv_sb[:], scores_sb[:])
            nc.vector.max_index(maxi_sb[:], maxv_sb[:], scores_sb[:])
            # compressed index c* = s* - (s* > t)   (faithful reference bug)
            nc.vector.tensor_copy(idxf_sb[:], maxi_sb[:, 0:K])
            nc.vector.tensor_scalar(gt_sb[:], idxf_sb[:], tcol_sb[:, 0:1], None, OP.is_gt)
            nc.vector.tensor_sub(cidxf_sb[:], idxf_sb[:], gt_sb[:])
            nc.vector.tensor_copy(cidx_sb[:], cidxf_sb[:])
            nc.sync.dma_start(row_sb[:], cidx_sb[:])
            # debug outputs
            nc.sync.dma_start(sdbg_h.ap(), scores_sb[:])
            nc.sync.dma_start(idbg_h.ap(), row_sb[:])

        # ================= Phase C: gather-combine + linear =================
        with tc.tile_pool(name="cps", bufs=4, space="PSUM") as cps, \
             tc.tile_pool(name="cpool", bufs=2) as cp:
            # batched register loads for all T*K gather offsets (<=32 per inst)
            avals = []
            half = T * K // 2
            for lo in range(0, T * K, half):
                _, v = nc.values_load_multi_w_load_instructions(
                    row_sb[0:1, lo:lo + half],
                    engines=bass.OrderedSet([mybir.EngineType.DVE]),
                    min_val=0, max_val=T - 2,
                    skip_runtime_bounds_check=True,
                )
                avals.extend(v)
            for t in range(T):
                vals = avals[K * t:K * (t + 1)]
                mf16 = cp.tile([128, CC, P], F16, tag="mf16")
                a0 = f16_sb[:, :, bass.ds(vals[0] * P, P)]
                a1 = f16_sb[:, :, bass.ds(vals[1] * P, P)]
                nc.vector.tensor_add(mf16[:], a0, a1)
                for k in range(2, K):
                    ak = f16_sb[:, :, bass.ds(vals[k] * P, P)]
                    nc.vector.tensor_add(mf16[:], mf16[:], ak)
                mf32 = cp.tile([128, CC, P], F32R, tag="mf32")
                nc.vector.tensor_copy(mf32[:], mf16[:])

                for dc in range(DC):
                    osb = cp.tile([128, P], FP32, tag="osb")
                    for ph in range(PH):
                        po = cps.tile([128, 512], FP32, tag="po")
                        for cc in range(CC):
                            # float32r: full-rate PE at ~fp32 precision
                            nc.tensor.matmul(
                                po[:],
                                wt3_sb[:, cc, dc * 128:(dc + 1) * 128],
                                mf32[:, cc, ph * 512:(ph + 1) * 512],
                                start=(cc == 0), stop=(cc == CC - 1),
                            )
                        nc.scalar.activation(osb[:, ph * 512:(ph + 1) * 512], po[:],
                                             AF.Identity, bias=bcol_sb[:, dc:dc + 1])
                    nc.sync.dma_start(out_h[t, dc * 128:(dc + 1) * 128, :], osb[:])


def build_program(T=16, C=256, P=1024, K=3):
    nc = bacc.Bacc("TRN2", target_bir_lowering=False, debug=False,
                   num_devices=N_CORES)
    handles = {}
    handles["features"] = nc.dram_tensor("features", [T, C, P], FP32,
                                         kind="ExternalInput")
    for name, shape, dt in (
        ("wt3", [128, C // 128, C], F32R),
        ("bcol", [128, C // 128], FP32),
        ("esel", [128, T * T], F16),
        ("i16", [T, T], FP32),
        ("diagbig", [T, T], FP32),
        ("tcol", [T, 1], FP32),
    ):
        handles[name] = nc.dram_tensor(name, shape, dt, kind="ExternalInput")
    handles["out"] = nc.dram_tensor("out", [T, C, P], FP32, kind="ExternalOutput")
    handles["scores_dbg"] = nc.dram_tensor("scores_dbg", [T, T], FP32,
                                           kind="ExternalOutput")
    handles["idx_dbg"] = nc.dram_tensor("idx_dbg", [1, T * K], I32,
                                        kind="ExternalOutput")

    with tile.TileContext(nc) as tc:
        _emit(nc, tc, T, C, P, K, handles)
    nc.compile()
    return nc


def _host_consts(W, b, T, C, K):
    consts = {}
    wt3 = (np.asarray(W, np.float32).T / float(K)).astype(np.float32)  # [C, C] (c, d)
    # SBUF tile is [c_in(partition), cc, d]
    consts["wt3"] = np.ascontiguousarray(
        wt3.reshape(C // 128, 128, C).transpose(1, 0, 2))
    consts["bcol"] = np.ascontiguousarray(
        np.asarray(b, np.float32).reshape(C // 128, 128).T)
    esel = np.zeros((128, T * T), np.float16)
    for t in range(T):
        esel[:, T * t + t] = 1.0
    consts["esel"] = esel
    consts["i16"] = np.eye(T, dtype=np.float32)
    consts["diagbig"] = (np.eye(T, dtype=np.float32) * BIG).astype(np.float32)
    consts["tcol"] = np.arange(T, dtype=np.float32).reshape(T, 1)
    return consts


_CACHE = {}


def kernel(features, W, b, top_k):
    features = np.asarray(features, np.float32)
    T, B, C, H, Wd = features.shape
    P = H * Wd
    K = int(top_k)
    assert B == N_CORES and C == 256 and P == 1024 and T == 16 and K == 3

    key = (T, C, P, K)
    if key not in _CACHE:
        _CACHE[key] = build_program(T, C, P, K)
    nc = _CACHE[key]

    consts = _host_consts(W, b, T, C, K)
    feat = features.reshape(T, B, C, P)
    in_maps = [
        {"features": np.ascontiguousarray(feat[:, i]), **consts}
        for i in range(N_CORES)
    ]
    res = run_bass_kernel_spmd(nc, in_maps, list(range(N_CORES)))
    out = np.stack([res.results[i]["out"] for i in range(N_CORES)], axis=1)
    return np.ascontiguousarray(out.reshape(T, B, C, H, Wd))


# revision 33
# speedup vs baseline: 1.8450x; 1.0111x over previous
"""Trainium2 Bass kernel for CrossFrameSimilarityRefiner.

Computation (per batch element b, fully batch-parallel -> B=8 sharded over 8 cores):
  f = features[:, b]                      # [T, C, P]  T=16, C=256, P=1024
  ss[t,p] = sum_c f^2 ; sm[t,p] = sum_c f ; gm[t,p] = sum_c (f>0)
  S[t,p]  = sm / sqrt(ss)                 # == sum/||.|| (eps clamp irrelevant for randn)
  M'[s,p] = gm  (affine transform of mean(sign(f)) -> identical per-row ranking)
  scores[t,s] = sum_p S[t,p] * M'[s,p]    # row-wise ranking == reference ranking
  mask diag, top-3 indices s* ; compressed c* = s* - (s* > t)   (reference's faithful bug:
  c* indexes the ORIGINAL frame axis)
  out[t] = (W/3) @ (f[c*0]+f[c*1]+f[c*2]) + b

Layout: features kept in SBUF c-major ([c_chunk partitions, t, p] free);
column reductions done on the PE via one-hot column-selector matmuls so each
t lands on its own PSUM partition; the 3-frame gather uses register-indexed
dynamic SBUF slices (bf16) and the final matmul runs in fp32.
"""

import numpy as np

import concourse.bacc as bacc
import concourse.bass as bass
import concourse.tile as tile
from concourse import mybir
from concourse.bass_utils import run_bass_kernel_spmd

FP32 = mybir.dt.float32
F32R = mybir.dt.float32r
F16 = mybir.dt.float16
I32 = mybir.dt.int32
U32 = mybir.dt.uint32
AF = mybir.ActivationFunctionType
OP = mybir.AluOpType

N_CORES = 8
BIG = 1.0e30


def _emit(nc, tc, T, C, P, K, handles):
    feat_h = handles["features"]
    out_h = handles["out"]
    sdbg_h = handles["scores_dbg"]
    idbg_h = handles["idx_dbg"]
    CC = C // 128          # c chunks (2)
    PH = P // 512          # psum-width chunks of p (2)
    PB = P // 128          # 128-blocks of p (8)
    DC = C // 128          # d chunks for output (2)

    with tc.tile_pool(name="persist", bufs=1) as pp:
        # ---- constants (from DRAM inputs) ----
        # W/K split into fp16 hi+lo halves: exact to ~2^-22 after accumulation
        wt3_sb = pp.tile([128, 2, CC, C], F16, tag="wt3")
        bcol_sb = pp.tile([128, DC], FP32, tag="bcol")
        esel_sb = pp.tile([128, T * T], F16, tag="esel")
        i16_sb = pp.tile([96, T], FP32, tag="i16")
        diag_sb = pp.tile([T, T], FP32, tag="diag")
        tcol_sb = pp.tile([T, 1], FP32, tag="tcol")
        for name, t_ in (("wt3", wt3_sb), ("bcol", bcol_sb), ("esel", esel_sb),
                         ("i16", i16_sb), ("diagbig", diag_sb), ("tcol", tcol_sb)):
            nc.sync.dma_start(t_[:], handles[name].ap())

        # ---- persistent state ----
        f16_sb = pp.tile([128, CC, T * P], F16, tag="f16")
        # stats rows: sm at partitions 0..15, ss at 32..47, gm at 64..79
        stats_sb = pp.tile([96, P], FP32, tag="stats")
        sm_sb = stats_sb[0:T, :]
        ss_sb = stats_sb[32:32 + T, :]
        gm_sb = stats_sb[64:64 + T, :]
        rn_sb = pp.tile([T, P], FP32, tag="rn")
        rs_sb = pp.tile([T, P], FP32, tag="rs")
        s_sb = pp.tile([T, P], FP32, tag="S")
        spt_sb = pp.tile([128, PB, T], FP32, tag="SpT")
        mpt_sb = pp.tile([128, PB, T], FP32, tag="MpT")
        scores_sb = pp.tile([T, T], FP32, tag="scores")
        maxv_sb = pp.tile([T, 8], FP32, tag="maxv")
        maxi_sb = pp.tile([T, 8], U32, tag="maxi")
        idxf_sb = pp.tile([T, K], FP32, tag="idxf")
        gt_sb = pp.tile([T, K], FP32, tag="gt")
        cidxf_sb = pp.tile([T, K], FP32, tag="cidxf")
        cidx_sb = pp.tile([T, K], I32, tag="cidx")
        row_sb = pp.tile([1, T * K], I32, tag="row")

        # ================= Phase A: stream in, stats =================
        with tc.tile_pool(name="statsps", bufs=1, space="PSUM") as sps, \
             tc.tile_pool(name="stream", bufs=4) as sp:
            # per (stat, p-half) psum banks; stat j writes partition strip 32j
            # so the 3 stats' matmuls col-tile and run concurrently on the PE
            st_ps = [[sps.tile([96, 512], FP32, tag=f"stp{ph}_{j}",
                               name=f"stp{ph}_{j}") for j in range(3)]
                     for ph in range(PH)]

            for t in range(T):
                for cc in range(CC):
                    fch = sp.tile([128, P], FP32, tag="fch")
                    nc.sync.dma_start(fch[:], feat_h[t, cc * 128:(cc + 1) * 128, :])
                    # fp16 copy: used by the stats matmuls AND the gather phase
                    f16c = f16_sb[:, cc, t * P:(t + 1) * P]
                    nc.vector.tensor_copy(f16c, fch[:])
                    sq = sp.tile([128, P], F16, tag="sq")
                    nc.scalar.activation(sq[:], fch[:], AF.Square)
                    gsc = sp.tile([128, P], F16, tag="gsc")
                    nc.vector.tensor_scalar(gsc[:], fch[:], 0.0, None, OP.is_gt)
                    st = (t == 0 and cc == 0)
                    sx = (t == T - 1 and cc == CC - 1)
                    lhs = esel_sb[:, T * t:T * (t + 1)]
                    for ph in range(PH):
                        sl = slice(ph * 512, (ph + 1) * 512)
                        for j, rhs in enumerate((f16c, sq[:], gsc[:])):
                            nc.tensor.matmul(
                                st_ps[ph][j][32 * j:32 * j + T, :], lhs, rhs[:, sl],
                                start=st, stop=sx, tile_position=(0, 32 * j))

            for ph in range(PH):
                for j in range(3):
                    nc.scalar.copy(
                        stats_sb[32 * j:32 * j + T, ph * 512:(ph + 1) * 512],
                        st_ps[ph][j][32 * j:32 * j + T, :])

        # ================= Phase B: scores + top-k =================
        with tc.tile_pool(name="bps", bufs=2, space="PSUM") as bps:
            nc.scalar.activation(rn_sb[:], ss_sb[:], AF.Sqrt)
            nc.vector.reciprocal(rs_sb[:], rn_sb[:])
            nc.vector.tensor_mul(s_sb[:], sm_sb[:], rs_sb[:])

            for src, ibase, dst in ((s_sb, 0, spt_sb), (gm_sb, 64, mpt_sb)):
                tr = bps.tile([128, PB * T], FP32, tag="tr", name="tr")
                ident = i16_sb[ibase:ibase + T, :]
                for pb in range(PB):
                    nc.tensor.transpose(tr[:, pb * T:(pb + 1) * T],
                                        src[:, pb * 128:(pb + 1) * 128], ident)
                nc.scalar.copy(dst[:, :, :], tr[:])

            sc_ps = bps.tile([T, T], FP32, tag="scps")
            for pb in range(PB):
                nc.tensor.matmul(sc_ps[:], spt_sb[:, pb, :], mpt_sb[:, pb, :],
                                 start=(pb == 0), stop=(pb == PB - 1))
            # exclude s == t, move to SBUF
            nc.vector.tensor_sub(scores_sb[:], sc_ps[:], diag_sb[:])

            nc.vector.max(maxv_sb[:], scores_sb[:])
            nc.vector.max_index(maxi_sb[:], maxv_sb[:], scores_sb[:])
            # compressed index c* = s* - (s* > t)   (faithful reference bug)
            nc.vector.tensor_copy(idxf_sb[:], maxi_sb[:, 0:K])
            nc.vector.tensor_scalar(gt_sb[:], idxf_sb[:], tcol_sb[:, 0:1], None, OP.is_gt)
            nc.vector.tensor_sub(cidxf_sb[:], idxf_sb[:], gt_sb[:])
            nc.vector.tensor_copy(cidx_sb[:], cidxf_sb[:])
            nc.sync.dma_start(row_sb[:], cidx_sb[:])
            # debug outputs
            nc.sync.dma_start(sdbg_h.ap(), scores_sb[:])
            nc.sync.dma_start(idbg_h.ap(), row_sb[:])

        # ================= Phase C: gather-combine + linear =================
        with tc.tile_pool(name="cps", bufs=4, space="PSUM") as cps, \
             tc.tile_pool(name="cpool", bufs=2) as cp:
            # batched register loads for all T*K gather offsets (<=32 per inst)
            avals = []
            half = T * K // 2
            for lo in range(0, T * K, half):
                _, v = nc.values_load_multi_w_load_instructions(
                    row_sb[0:1, lo:lo + half],
                    engines=bass.OrderedSet([mybir.EngineType.DVE]),
                    min_val=0, max_val=T - 2,
                    skip_runtime_bounds_check=True,
                )
                avals.extend(v)
            for t in range(T):
                vals = avals[K * t:K * (t + 1)]
                mf16 = cp.tile([128, CC, P], F16, tag="mf16")
                a0 = f16_sb[:, :, bass.ds(vals[0] * P, P)]
                a1 = f16_sb[:, :, bass.ds(vals[1] * P, P)]
                nc.vector.tensor_add(mf16[:], a0, a1)
                for k in range(2, K):
                    ak = f16_sb[:, :, bass.ds(vals[k] * P, P)]
                    nc.vector.tensor_add(mf16[:], mf16[:], ak)
                for dc in range(DC):
                    osb = cp.tile([128, P], FP32, tag="osb")
                    for ph in range(PH):
                        po = cps.tile([128, 512], FP32, tag="po")
                        for s in range(2):
                            for cc in range(CC):
                                nc.tensor.matmul(
                                    po[:],
                                    wt3_sb[:, s, cc, dc * 128:(dc + 1) * 128],
                                    mf16[:, cc, ph * 512:(ph + 1) * 512],
                                    start=(s == 0 and cc == 0),
                                    stop=(s == 1 and cc == CC - 1),
                                )
                        nc.scalar.activation(osb[:, ph * 512:(ph + 1) * 512], po[:],
                                             AF.Identity, bias=bcol_sb[:, dc:dc + 1])
                    nc.sync.dma_start(out_h[t, dc * 128:(dc + 1) * 128, :], osb[:])


def build_program(T=16, C=256, P=1024, K=3):
    nc = bacc.Bacc("TRN2", target_bir_lowering=False, debug=False,
                   num_devices=N_CORES)
    handles = {}
    handles["features"] = nc.dram_tensor("features", [T, C, P], FP32,
                                         kind="ExternalInput")
    for name, shape, dt in (
        ("wt3", [128, 2, C // 128, C], F16),
        ("bcol", [128, C // 128], FP32),
        ("esel", [128, T * T], F16),
        ("i16", [96, T], FP32),
        ("diagbig", [T, T], FP32),
        ("tcol", [T, 1], FP32),
    ):
        handles[name] = nc.dram_tensor(name, shape, dt, kind="ExternalInput")
    handles["out"] = nc.dram_tensor("out", [T, C, P], FP32, kind="ExternalOutput")
    handles["scores_dbg"] = nc.dram_tensor("scores_dbg", [T, T], FP32,
                                           kind="ExternalOutput")
    handles["idx_dbg"] = nc.dram_tensor("idx_dbg", [1, T * K], I32,
                                        kind="ExternalOutput")

    with tile.TileContext(nc) as tc:
        _emit(nc, tc, T, C, P, K, handles)
    nc.compile()
    return nc


def _host_consts(W, b, T, C, K):
    consts = {}
    wt3 = (np.asarray(W, np.float32).T / float(K)).astype(np.float32)  # [C, C] (c, d)
    # [c_in(partition), split, cc, d]: fp16 hi + fp16 residual
    w4 = wt3.reshape(C // 128, 128, C).transpose(1, 0, 2)  # [128, cc, d]
    whi = w4.astype(np.float16)
    wlo = (w4 - whi.astype(np.float32)).astype(np.float16)
    consts["wt3"] = np.ascontiguousarray(
        np.stack([whi, wlo], axis=1))  # [128, 2, cc, d]
    consts["bcol"] = np.ascontiguousarray(
        np.asarray(b, np.float32).reshape(C // 128, 128).T)
    esel = np.zeros((128, T * T), np.float16)
    for t in range(T):
        esel[:, T * t + t] = 1.0
    consts["esel"] = esel
    i16 = np.zeros((96, T), np.float32)
    i16[0:T, :] = np.eye(T, dtype=np.float32)
    i16[64:64 + T, :] = np.eye(T, dtype=np.float32)
    consts["i16"] = i16
    consts["diagbig"] = (np.eye(T, dtype=np.float32) * BIG).astype(np.float32)
    consts["tcol"] = np.arange(T, dtype=np.float32).reshape(T, 1)
    return consts


_CACHE = {}


def kernel(features, W, b, top_k):
    features = np.asarray(features, np.float32)
    T, B, C, H, Wd = features.shape
    P = H * Wd
    K = int(top_k)
    assert B == N_CORES and C == 256 and P == 1024 and T == 16 and K == 3

    key = (T, C, P, K)
    if key not in _CACHE:
        _CACHE[key] = build_program(T, C, P, K)
    nc = _CACHE[key]

    consts = _host_consts(W, b, T, C, K)
    feat = features.reshape(T, B, C, P)
    in_maps = [
        {"features": np.ascontiguousarray(feat[:, i]), **consts}
        for i in range(N_CORES)
    ]
    res = run_bass_kernel_spmd(nc, in_maps, list(range(N_CORES)))
    out = np.stack([res.results[i]["out"] for i in range(N_CORES)], axis=1)
    return np.ascontiguousarray(out.reshape(T, B, C, H, Wd))


# revision 41
# speedup vs baseline: 1.9657x; 1.0654x over previous
"""Trainium2 Bass kernel for CrossFrameSimilarityRefiner.

Computation (per batch element b, fully batch-parallel -> B=8 sharded over 8 cores):
  f = features[:, b]                      # [T, C, P]  T=16, C=256, P=1024
  ss[t,p] = sum_c f^2 ; sm[t,p] = sum_c f ; gm[t,p] = sum_c (f>0)
  S[t,p]  = sm / sqrt(ss)                 # == sum/||.|| (eps clamp irrelevant for randn)
  M'[s,p] = gm  (affine transform of mean(sign(f)) -> identical per-row ranking)
  scores[t,s] = sum_p S[t,p] * M'[s,p]    # row-wise ranking == reference ranking
  mask diag, top-3 indices s* ; compressed c* = s* - (s* > t)   (reference's faithful bug:
  c* indexes the ORIGINAL frame axis)
  out[t] = (W/3) @ (f[c*0]+f[c*1]+f[c*2]) + b

Layout: features kept in SBUF c-major ([c_chunk partitions, t, p] free);
column reductions done on the PE via one-hot column-selector matmuls so each
t lands on its own PSUM partition; the 3-frame gather uses register-indexed
dynamic SBUF slices (bf16) and the final matmul runs in fp32.
"""

import numpy as np

import concourse.bacc as bacc
import concourse.bass as bass
import concourse.tile as tile
from concourse import mybir
from concourse.bass_utils import run_bass_kernel_spmd

FP32 = mybir.dt.float32
F32R = mybir.dt.float32r
F16 = mybir.dt.float16
I32 = mybir.dt.int32
U32 = mybir.dt.uint32
AF = mybir.ActivationFunctionType
OP = mybir.AluOpType

N_CORES = 8
BIG = 1.0e30


def _emit(nc, tc, T, C, P, K, handles):
    feat_h = handles["features"]
    out_h = handles["out"]
    sdbg_h = handles["scores_dbg"]
    idbg_h = handles["idx_dbg"]
    CC = C // 128          # c chunks (2)
    PH = P // 512          # psum-width chunks of p (2)
    PB = P // 128          # 128-blocks of p (8)
    DC = C // 128          # d chunks for output (2)

    with tc.tile_pool(name="persist", bufs=1) as pp:
        # ---- constants (from DRAM inputs) ----
        wt3_sb = pp.tile([128, CC, C], F16, tag="wt3")
        bcol_sb = pp.tile([128, DC], FP32, tag="bcol")
        esel_sb = pp.tile([128, T * T], F16, tag="esel")
        i16_sb = pp.tile([96, T], FP32, tag="i16")
        diag_sb = pp.tile([T, T], FP32, tag="diag")
        tcol_sb = pp.tile([T, 1], FP32, tag="tcol")
        for name, t_ in (("wt3", wt3_sb), ("bcol", bcol_sb), ("esel", esel_sb),
                         ("i16", i16_sb), ("diagbig", diag_sb), ("tcol", tcol_sb)):
            nc.sync.dma_start(t_[:], handles[name].ap())

        # ---- persistent state ----
        f16_sb = pp.tile([128, CC, T * P], F16, tag="f16")
        # stats rows: sm at partitions 0..15, ss at 32..47, gm at 64..79
        stats_sb = pp.tile([96, P], FP32, tag="stats")
        sm_sb = stats_sb[0:T, :]
        ss_sb = stats_sb[32:32 + T, :]
        gm_sb = stats_sb[64:64 + T, :]
        smt_sb = pp.tile([128, PB, T], FP32, tag="smT")
        rnt_sb = pp.tile([128, PB, T], FP32, tag="rnT")
        rst_sb = pp.tile([128, PB, T], FP32, tag="rsT")
        spt_sb = pp.tile([128, PB, T], FP32, tag="SpT")
        mpt_sb = pp.tile([128, PB, T], FP32, tag="MpT")
        scores_sb = pp.tile([T, T], FP32, tag="scores")
        maxv_sb = pp.tile([T, 8], FP32, tag="maxv")
        maxi_sb = pp.tile([T, 8], U32, tag="maxi")
        idxf_sb = pp.tile([T, K], FP32, tag="idxf")
        gt_sb = pp.tile([T, K], FP32, tag="gt")
        cidxf_sb = pp.tile([T, K], FP32, tag="cidxf")
        cidx_sb = pp.tile([T, K], I32, tag="cidx")
        row_sb = pp.tile([1, T * K], I32, tag="row")

        # ================= Phase A: stream in, stats =================
        with tc.tile_pool(name="statsps", bufs=1, space="PSUM") as sps, \
             tc.tile_pool(name="stream", bufs=4) as sp:
            # per (stat, p-half) psum banks; stat j writes partition strip 32j
            # so the 3 stats' matmuls col-tile and run concurrently on the PE
            st_ps = [[sps.tile([96, 512], FP32, tag=f"stp{ph}_{j}",
                               name=f"stp{ph}_{j}") for j in range(3)]
                     for ph in range(PH)]

            last_sq = None
            for t in range(T):
                fch = sp.tile([128, CC, P], FP32, tag="fch")
                for cc in range(CC):
                    nc.sync.dma_start(fch[:, cc, :],
                                      feat_h[t, cc * 128:(cc + 1) * 128, :])
                # fp16 copy: used by the stats matmuls AND the gather phase
                f16c = f16_sb[:, :, t * P:(t + 1) * P]
                nc.vector.tensor_copy(f16c, fch[:])
                sq = sp.tile([128, CC, P], F16, tag="sq")
                nc.scalar.activation(sq[:], fch[:], AF.Square)
                last_sq = sq
                gsc = sp.tile([128, CC, P], F16, tag="gsc")
                nc.vector.tensor_scalar(gsc[:], fch[:], 0.0, None, OP.is_gt)
                st = (t == 0)
                sx = (t == T - 1)
                lhs = esel_sb[:, T * t:T * (t + 1)]
                for cc in range(CC):
                    for ph in range(PH):
                        sl = slice(ph * 512, (ph + 1) * 512)
                        for j, src in enumerate((f16c, sq, gsc)):
                            nc.tensor.matmul(
                                st_ps[ph][j][32 * j:32 * j + T, :], lhs,
                                src[:, cc, sl],
                                start=st and cc == 0, stop=sx and cc == CC - 1,
                                tile_position=(0, 32 * j))

            # preload the Sqrt ACT table while the stats evacuate (depends on
            # the last Square so it cannot run early and evict its table)
            dummy_sb = sp.tile([1, 1], FP32, tag="dummy")
            nc.scalar.activation(dummy_sb[:], last_sq[0:1, 0, 0:1], AF.Sqrt)

            for ph in range(PH):
                for j in range(3):
                    nc.scalar.copy(
                        stats_sb[32 * j:32 * j + T, ph * 512:(ph + 1) * 512],
                        st_ps[ph][j][32 * j:32 * j + T, :])

        # ================= Phase B: scores + top-k =================
        with tc.tile_pool(name="bps", bufs=2, space="PSUM") as bps:
            # transpose raw stats to p-major first; the sqrt/recip/mul then
            # run on all 128 partitions instead of 16
            for src, ibase, dst in ((sm_sb, 0, smt_sb), (ss_sb, 32, rnt_sb),
                                    (gm_sb, 64, mpt_sb)):
                tr = bps.tile([128, PB * T], FP32, tag="tr", name="tr")
                ident = i16_sb[ibase:ibase + T, :]
                for pb in range(PB):
                    nc.tensor.transpose(tr[:, pb * T:(pb + 1) * T],
                                        src[:, pb * 128:(pb + 1) * 128], ident)
                nc.scalar.copy(dst[:, :, :], tr[:])
            # rnT <- sqrt(ssT) in place, then rsT = 1/rnT, S_pT = smT*rsT
            nc.scalar.activation(rnt_sb[:], rnt_sb[:], AF.Sqrt)
            nc.vector.reciprocal(rst_sb[:], rnt_sb[:])
            nc.vector.tensor_mul(spt_sb[:], smt_sb[:], rst_sb[:])

            sc_ps = bps.tile([T, T], FP32, tag="scps")
            for pb in range(PB):
                nc.tensor.matmul(sc_ps[:], spt_sb[:, pb, :], mpt_sb[:, pb, :],
                                 start=(pb == 0), stop=(pb == PB - 1))
            # exclude s == t, move to SBUF
            nc.vector.tensor_sub(scores_sb[:], sc_ps[:], diag_sb[:])

            nc.vector.max(maxv_sb[:], scores_sb[:])
            nc.vector.max_index(maxi_sb[:], maxv_sb[:], scores_sb[:])
            # compressed index c* = s* - (s* > t)   (faithful reference bug)
            nc.vector.tensor_copy(idxf_sb[:], maxi_sb[:, 0:K])
            nc.vector.tensor_scalar(gt_sb[:], idxf_sb[:], tcol_sb[:, 0:1], None, OP.is_gt)
            nc.vector.tensor_sub(cidxf_sb[:], idxf_sb[:], gt_sb[:])
            nc.vector.tensor_copy(cidx_sb[:], cidxf_sb[:])
            nc.sync.dma_start(row_sb[:], cidx_sb[:])
            # debug outputs
            nc.sync.dma_start(sdbg_h.ap(), scores_sb[:])
            nc.sync.dma_start(idbg_h.ap(), row_sb[:])

        # ================= Phase C: gather-combine + linear =================
        with tc.tile_pool(name="cps", bufs=4, space="PSUM") as cps, \
             tc.tile_pool(name="cpool", bufs=2) as cp:
            # batched register loads for all T*K gather offsets (<=32 per inst)
            avals = []
            half = T * K // 2
            for lo in range(0, T * K, half):
                _, v = nc.values_load_multi_w_load_instructions(
                    row_sb[0:1, lo:lo + half],
                    engines=bass.OrderedSet([mybir.EngineType.DVE]),
                    min_val=0, max_val=T - 2,
                    skip_runtime_bounds_check=True,
                )
                avals.extend(v)
            for t in range(T):
                vals = avals[K * t:K * (t + 1)]
                mf16 = cp.tile([128, CC, P], F16, tag="mf16")
                a0 = f16_sb[:, :, bass.ds(vals[0] * P, P)]
                a1 = f16_sb[:, :, bass.ds(vals[1] * P, P)]
                nc.vector.tensor_add(mf16[:], a0, a1)
                for k in range(2, K):
                    ak = f16_sb[:, :, bass.ds(vals[k] * P, P)]
                    nc.vector.tensor_add(mf16[:], mf16[:], ak)
                for dc in range(DC):
                    osb = cp.tile([128, P], FP32, tag="osb")
                    for ph in range(PH):
                        po = cps.tile([128, 512], FP32, tag="po")
                        for cc in range(CC):
                            nc.tensor.matmul(
                                po[:],
                                wt3_sb[:, cc, dc * 128:(dc + 1) * 128],
                                mf16[:, cc, ph * 512:(ph + 1) * 512],
                                start=(cc == 0), stop=(cc == CC - 1),
                            )
                        nc.scalar.activation(osb[:, ph * 512:(ph + 1) * 512], po[:],
                                             AF.Identity, bias=bcol_sb[:, dc:dc + 1])
                    nc.sync.dma_start(out_h[t, dc * 128:(dc + 1) * 128, :], osb[:])


def build_program(T=16, C=256, P=1024, K=3):
    nc = bacc.Bacc("TRN2", target_bir_lowering=False, debug=False,
                   num_devices=N_CORES)
    handles = {}
    handles["features"] = nc.dram_tensor("features", [T, C, P], FP32,
                                         kind="ExternalInput")
    for name, shape, dt in (
        ("wt3", [128, C // 128, C], F16),
        ("bcol", [128, C // 128], FP32),
        ("esel", [128, T * T], F16),
        ("i16", [96, T], FP32),
        ("diagbig", [T, T], FP32),
        ("tcol", [T, 1], FP32),
    ):
        handles[name] = nc.dram_tensor(name, shape, dt, kind="ExternalInput")
    handles["out"] = nc.dram_tensor("out", [T, C, P], FP32, kind="ExternalOutput")
    handles["scores_dbg"] = nc.dram_tensor("scores_dbg", [T, T], FP32,
                                           kind="ExternalOutput")
    handles["idx_dbg"] = nc.dram_tensor("idx_dbg", [1, T * K], I32,
                                        kind="ExternalOutput")

    with tile.TileContext(nc) as tc:
        _emit(nc, tc, T, C, P, K, handles)
    nc.compile()
    return nc


def _host_consts(W, b, T, C, K):
    consts = {}
    wt3 = (np.asarray(W, np.float32).T / float(K)).astype(np.float32)  # [C, C] (c, d)
    # [c_in(partition), cc, d] in fp16
    w4 = wt3.reshape(C // 128, 128, C).transpose(1, 0, 2)
    consts["wt3"] = np.ascontiguousarray(w4.astype(np.float16))
    consts["bcol"] = np.ascontiguousarray(
        np.asarray(b, np.float32).reshape(C // 128, 128).T)
    esel = np.zeros((128, T * T), np.float16)
    for t in range(T):
        esel[:, T * t + t] = 1.0
    consts["esel"] = esel
    i16 = np.zeros((96, T), np.float32)
    for r in (0, 32, 64):
        i16[r:r + T, :] = np.eye(T, dtype=np.float32)
    consts["i16"] = i16
    consts["diagbig"] = (np.eye(T, dtype=np.float32) * BIG).astype(np.float32)
    consts["tcol"] = np.arange(T, dtype=np.float32).reshape(T, 1)
    return consts


_CACHE = {}


def kernel(features, W, b, top_k):
    features = np.asarray(features, np.float32)
    T, B, C, H, Wd = features.shape
    P = H * Wd
    K = int(top_k)
    assert B == N_CORES and C == 256 and P == 1024 and T == 16 and K == 3

    key = (T, C, P, K)
    if key not in _CACHE:
        _CACHE[key] = build_program(T, C, P, K)
    nc = _CACHE[key]

    consts = _host_consts(W, b, T, C, K)
    feat = features.reshape(T, B, C, P)
    in_maps = [
        {"features": np.ascontiguousarray(feat[:, i]), **consts}
        for i in range(N_CORES)
    ]
    res = run_bass_kernel_spmd(nc, in_maps, list(range(N_CORES)))
    out = np.stack([res.results[i]["out"] for i in range(N_CORES)], axis=1)
    return np.ascontiguousarray(out.reshape(T, B, C, H, Wd))


# revision 43
# speedup vs baseline: 2.5535x; 1.2990x over previous
"""Trainium2 Bass kernel for CrossFrameSimilarityRefiner.

Computation (per batch element b, fully batch-parallel -> B=8 sharded over 8 cores):
  f = features[:, b]                      # [T, C, P]  T=16, C=256, P=1024
  ss[t,p] = sum_c f^2 ; sm[t,p] = sum_c f ; gm[t,p] = sum_c (f>0)
  S[t,p]  = sm / sqrt(ss)                 # == sum/||.|| (eps clamp irrelevant for randn)
  M'[s,p] = gm  (affine transform of mean(sign(f)) -> identical per-row ranking)
  scores[t,s] = sum_p S[t,p] * M'[s,p]    # row-wise ranking == reference ranking
  mask diag, top-3 indices s* ; compressed c* = s* - (s* > t)   (reference's faithful bug:
  c* indexes the ORIGINAL frame axis)
  out[t] = (W/3) @ (f[c*0]+f[c*1]+f[c*2]) + b

Layout: features kept in SBUF c-major ([c_chunk partitions, t, p] free);
column reductions done on the PE via one-hot column-selector matmuls so each
t lands on its own PSUM partition; the 3-frame gather uses register-indexed
dynamic SBUF slices (bf16) and the final matmul runs in fp32.
"""

import numpy as np

import concourse.bacc as bacc
import concourse.bass as bass
import concourse.tile as tile
from concourse import mybir
from concourse.bass_utils import run_bass_kernel_spmd

FP32 = mybir.dt.float32
F32R = mybir.dt.float32r
F16 = mybir.dt.float16
I32 = mybir.dt.int32
U32 = mybir.dt.uint32
AF = mybir.ActivationFunctionType
OP = mybir.AluOpType

N_CORES = 8
BIG = 1.0e30


def _emit(nc, tc, T, C, P, K, handles):
    feat_h = handles["features"]
    out_h = handles["out"]
    sdbg_h = handles["scores_dbg"]
    idbg_h = handles["idx_dbg"]
    CC = C // 128          # c chunks (2)
    PH = P // 512          # psum-width chunks of p (2)
    PB = P // 128          # 128-blocks of p (8)
    DC = C // 128          # d chunks for output (2)

    with tc.tile_pool(name="persist", bufs=1) as pp:
        # ---- constants (from DRAM inputs) ----
        wt3_sb = pp.tile([128, CC, C], F16, tag="wt3")
        bcol_sb = pp.tile([128, DC], FP32, tag="bcol")
        esel_sb = pp.tile([128, T * T], F16, tag="esel")
        i16_sb = pp.tile([96, T], FP32, tag="i16")
        diag_sb = pp.tile([T, T], FP32, tag="diag")
        tcol_sb = pp.tile([T, 1], FP32, tag="tcol")
        for name, t_ in (("wt3", wt3_sb), ("bcol", bcol_sb), ("esel", esel_sb),
                         ("i16", i16_sb), ("diagbig", diag_sb), ("tcol", tcol_sb)):
            nc.sync.dma_start(t_[:], handles[name].ap())

        # ---- persistent state ----
        f16_sb = pp.tile([128, CC, T * P], F16, tag="f16")
        # stats rows: sm at partitions 0..15, ss at 32..47, gm at 64..79
        stats_sb = pp.tile([96, P], FP32, tag="stats")
        sm_sb = stats_sb[0:T, :]
        ss_sb = stats_sb[32:32 + T, :]
        gm_sb = stats_sb[64:64 + T, :]
        smt_sb = pp.tile([128, PB, T], FP32, tag="smT")
        rnt_sb = pp.tile([128, PB, T], FP32, tag="rnT")
        rst_sb = pp.tile([128, PB, T], FP32, tag="rsT")
        spt_sb = pp.tile([128, PB, T], FP32, tag="SpT")
        mpt_sb = pp.tile([128, PB, T], FP32, tag="MpT")
        scores_sb = pp.tile([T, T], FP32, tag="scores")
        maxv_sb = pp.tile([T, 8], FP32, tag="maxv")
        maxi_sb = pp.tile([T, 8], U32, tag="maxi")
        idxf_sb = pp.tile([T, K], FP32, tag="idxf")
        gt_sb = pp.tile([T, K], FP32, tag="gt")
        cidxf_sb = pp.tile([T, K], FP32, tag="cidxf")
        cidx_sb = pp.tile([T, K], I32, tag="cidx")
        row_sb = pp.tile([1, T * K], I32, tag="row")

        # ================= Phase A: stream in, stats =================
        with tc.tile_pool(name="statsps", bufs=1, space="PSUM") as sps, \
             tc.tile_pool(name="stream", bufs=4) as sp:
            # per (stat, p-half) psum banks; stat j writes partition strip 32j
            # so the 3 stats' matmuls col-tile and run concurrently on the PE
            st_ps = [[sps.tile([96, 512], FP32, tag=f"stp{ph}_{j}",
                               name=f"stp{ph}_{j}") for j in range(3)]
                     for ph in range(PH)]

            last_sq = None
            for t in range(T):
                fch = sp.tile([128, CC, P], FP32, tag="fch")
                for cc in range(CC):
                    nc.sync.dma_start(fch[:, cc, :],
                                      feat_h[t, cc * 128:(cc + 1) * 128, :])
                # fp16 copy: used by the stats matmuls AND the gather phase
                f16c = f16_sb[:, :, t * P:(t + 1) * P]
                nc.vector.tensor_copy(f16c, fch[:])
                sq = sp.tile([128, CC, P], F16, tag="sq")
                nc.scalar.activation(sq[:], fch[:], AF.Square)
                last_sq = sq
                gsc = sp.tile([128, CC, P], F16, tag="gsc")
                nc.vector.tensor_scalar(gsc[:], fch[:], 0.0, None, OP.is_gt)
                st = (t == 0)
                sx = (t == T - 1)
                lhs = esel_sb[:, T * t:T * (t + 1)]
                for cc in range(CC):
                    for ph in range(PH):
                        sl = slice(ph * 512, (ph + 1) * 512)
                        for j, src in enumerate((f16c, sq, gsc)):
                            nc.tensor.matmul(
                                st_ps[ph][j][32 * j:32 * j + T, :], lhs,
                                src[:, cc, sl],
                                start=st and cc == 0, stop=sx and cc == CC - 1,
                                tile_position=(0, 32 * j))

            # preload the Sqrt ACT table while the stats evacuate (depends on
            # the last Square so it cannot run early and evict its table)
            dummy_sb = sp.tile([1, 1], FP32, tag="dummy")
            nc.scalar.activation(dummy_sb[:], last_sq[0:1, 0, 0:1], AF.Sqrt)

            for ph in range(PH):
                for j in range(3):
                    nc.scalar.copy(
                        stats_sb[32 * j:32 * j + T, ph * 512:(ph + 1) * 512],
                        st_ps[ph][j][32 * j:32 * j + T, :])

        # ================= Phase B: scores + top-k =================
        with tc.tile_pool(name="bps", bufs=2, space="PSUM") as bps:
            # transpose raw stats to p-major first; the sqrt/recip/mul then
            # run on all 128 partitions instead of 16
            for src, ibase, dst in ((sm_sb, 0, smt_sb), (ss_sb, 32, rnt_sb),
                                    (gm_sb, 64, mpt_sb)):
                tr = bps.tile([128, PB * T], FP32, tag="tr", name="tr")
                ident = i16_sb[ibase:ibase + T, :]
                for pb in range(PB):
                    nc.tensor.transpose(tr[:, pb * T:(pb + 1) * T],
                                        src[:, pb * 128:(pb + 1) * 128], ident)
                nc.scalar.copy(dst[:, :, :], tr[:])
            # rnT <- sqrt(ssT) in place, then rsT = 1/rnT, S_pT = smT*rsT
            nc.scalar.activation(rnt_sb[:], rnt_sb[:], AF.Sqrt)
            nc.vector.reciprocal(rst_sb[:], rnt_sb[:])
            nc.vector.tensor_mul(spt_sb[:], smt_sb[:], rst_sb[:])

            sc_ps = bps.tile([T, T], FP32, tag="scps")
            for pb in range(PB):
                nc.tensor.matmul(sc_ps[:], spt_sb[:, pb, :], mpt_sb[:, pb, :],
                                 start=(pb == 0), stop=(pb == PB - 1))
            # exclude s == t, move to SBUF
            nc.vector.tensor_sub(scores_sb[:], sc_ps[:], diag_sb[:])

            nc.vector.max(maxv_sb[:], scores_sb[:])
            nc.vector.max_index(maxi_sb[:], maxv_sb[:], scores_sb[:])
            # compressed index c* = s* - (s* > t)   (faithful reference bug)
            nc.vector.tensor_copy(idxf_sb[:], maxi_sb[:, 0:K])
            nc.vector.tensor_scalar(gt_sb[:], idxf_sb[:], tcol_sb[:, 0:1], None, OP.is_gt)
            nc.vector.tensor_sub(cidxf_sb[:], idxf_sb[:], gt_sb[:])
            nc.vector.tensor_copy(cidx_sb[:], cidxf_sb[:])
            nc.sync.dma_start(row_sb[:], cidx_sb[:])
            # debug outputs
            nc.sync.dma_start(sdbg_h.ap(), scores_sb[:])
            nc.sync.dma_start(idbg_h.ap(), row_sb[:])

        # ================= Phase C: gather-combine + linear =================
        with tc.tile_pool(name="cps", bufs=4, space="PSUM") as cps, \
             tc.tile_pool(name="cpool", bufs=3) as cp:
            # batched register loads for all T*K gather offsets (<=32 per inst)
            avals = []
            half = T * K // 2
            for lo in range(0, T * K, half):
                _, v = nc.values_load_multi_w_load_instructions(
                    row_sb[0:1, lo:lo + half],
                    engines=bass.OrderedSet([mybir.EngineType.DVE]),
                    min_val=0, max_val=T - 2,
                    skip_runtime_bounds_check=True,
                )
                avals.extend(v)
            for t in range(T):
                vals = avals[K * t:K * (t + 1)]
                mf16 = cp.tile([128, CC, P], F16, tag="mf16")
                a0 = f16_sb[:, :, bass.ds(vals[0] * P, P)]
                a1 = f16_sb[:, :, bass.ds(vals[1] * P, P)]
                nc.vector.tensor_add(mf16[:], a0, a1)
                for k in range(2, K):
                    ak = f16_sb[:, :, bass.ds(vals[k] * P, P)]
                    nc.vector.tensor_add(mf16[:], mf16[:], ak)
                for dc in range(DC):
                    osb = cp.tile([128, P], FP32, tag="osb", bufs=4)
                    # [128,1024] psum tile spans 2 banks; each 512-half is its
                    # own accumulation group -> one wide IDENTITY evac per dc
                    po = cps.tile([128, P], FP32, tag="po")
                    for ph in range(PH):
                        for cc in range(CC):
                            nc.tensor.matmul(
                                po[:, ph * 512:(ph + 1) * 512],
                                wt3_sb[:, cc, dc * 128:(dc + 1) * 128],
                                mf16[:, cc, ph * 512:(ph + 1) * 512],
                                start=(cc == 0), stop=(cc == CC - 1),
                            )
                    nc.scalar.activation(osb[:], po[:],
                                         AF.Identity, bias=bcol_sb[:, dc:dc + 1])
                    nc.sync.dma_start(out_h[t, dc * 128:(dc + 1) * 128, :], osb[:])


def build_program(T=16, C=256, P=1024, K=3):
    nc = bacc.Bacc("TRN2", target_bir_lowering=False, debug=False,
                   num_devices=N_CORES)
    handles = {}
    handles["features"] = nc.dram_tensor("features", [T, C, P], FP32,
                                         kind="ExternalInput")
    for name, shape, dt in (
        ("wt3", [128, C // 128, C], F16),
        ("bcol", [128, C // 128], FP32),
        ("esel", [128, T * T], F16),
        ("i16", [96, T], FP32),
        ("diagbig", [T, T], FP32),
        ("tcol", [T, 1], FP32),
    ):
        handles[name] = nc.dram_tensor(name, shape, dt, kind="ExternalInput")
    handles["out"] = nc.dram_tensor("out", [T, C, P], FP32, kind="ExternalOutput")
    handles["scores_dbg"] = nc.dram_tensor("scores_dbg", [T, T], FP32,
                                           kind="ExternalOutput")
    handles["idx_dbg"] = nc.dram_tensor("idx_dbg", [1, T * K], I32,
                                        kind="ExternalOutput")

    with tile.TileContext(nc) as tc:
        _emit(nc, tc, T, C, P, K, handles)
    nc.compile()
    return nc


def _host_consts(W, b, T, C, K):
    consts = {}
    wt3 = (np.asarray(W, np.float32).T / float(K)).astype(np.float32)  # [C, C] (c, d)
    # [c_in(partition), cc, d] in fp16
    w4 = wt3.reshape(C // 128, 128, C).transpose(1, 0, 2)
    consts["wt3"] = np.ascontiguousarray(w4.astype(np.float16))
    consts["bcol"] = np.ascontiguousarray(
        np.asarray(b, np.float32).reshape(C // 128, 128).T)
    esel = np.zeros((128, T * T), np.float16)
    for t in range(T):
        esel[:, T * t + t] = 1.0
    consts["esel"] = esel
    i16 = np.zeros((96, T), np.float32)
    for r in (0, 32, 64):
        i16[r:r + T, :] = np.eye(T, dtype=np.float32)
    consts["i16"] = i16
    consts["diagbig"] = (np.eye(T, dtype=np.float32) * BIG).astype(np.float32)
    consts["tcol"] = np.arange(T, dtype=np.float32).reshape(T, 1)
    return consts


_CACHE = {}


def kernel(features, W, b, top_k):
    features = np.asarray(features, np.float32)
    T, B, C, H, Wd = features.shape
    P = H * Wd
    K = int(top_k)
    assert B == N_CORES and C == 256 and P == 1024 and T == 16 and K == 3

    key = (T, C, P, K)
    if key not in _CACHE:
        _CACHE[key] = build_program(T, C, P, K)
    nc = _CACHE[key]

    consts = _host_consts(W, b, T, C, K)
    feat = features.reshape(T, B, C, P)
    in_maps = [
        {"features": np.ascontiguousarray(feat[:, i]), **consts}
        for i in range(N_CORES)
    ]
    res = run_bass_kernel_spmd(nc, in_maps, list(range(N_CORES)))
    out = np.stack([res.results[i]["out"] for i in range(N_CORES)], axis=1)
    return np.ascontiguousarray(out.reshape(T, B, C, H, Wd))


# revision 48
# speedup vs baseline: 2.7274x; 1.0681x over previous
"""Trainium2 Bass kernel for CrossFrameSimilarityRefiner.

Computation (per batch element b, fully batch-parallel -> B=8 sharded over 8 cores):
  f = features[:, b]                      # [T, C, P]  T=16, C=256, P=1024
  ss[t,p] = sum_c f^2 ; sm[t,p] = sum_c f ; gm[t,p] = sum_c (f>0)
  S[t,p]  = sm / sqrt(ss)                 # == sum/||.|| (eps clamp irrelevant for randn)
  M'[s,p] = gm  (affine transform of mean(sign(f)) -> identical per-row ranking)
  scores[t,s] = sum_p S[t,p] * M'[s,p]    # row-wise ranking == reference ranking
  mask diag, top-3 indices s* ; compressed c* = s* - (s* > t)   (reference's faithful bug:
  c* indexes the ORIGINAL frame axis)
  out[t] = (W/3) @ (f[c*0]+f[c*1]+f[c*2]) + b

Layout: features kept in SBUF c-major ([c_chunk partitions, t, p] free);
column reductions done on the PE via one-hot column-selector matmuls so each
t lands on its own PSUM partition; the 3-frame gather uses register-indexed
dynamic SBUF slices (bf16) and the final matmul runs in fp32.
"""

import numpy as np

import concourse.bacc as bacc
import concourse.bass as bass
import concourse.tile as tile
from concourse import mybir
from concourse.bass_utils import run_bass_kernel_spmd

FP32 = mybir.dt.float32
F32R = mybir.dt.float32r
F16 = mybir.dt.float16
I32 = mybir.dt.int32
U32 = mybir.dt.uint32
AF = mybir.ActivationFunctionType
OP = mybir.AluOpType

N_CORES = 8
BIG = 1.0e30


def _emit(nc, tc, T, C, P, K, handles):
    feat_h = handles["features"]
    out_h = handles["out"]
    sdbg_h = handles["scores_dbg"]
    idbg_h = handles["idx_dbg"]
    CC = C // 128          # c chunks (2)
    PH = P // 512          # psum-width chunks of p (2)
    PB = P // 128          # 128-blocks of p (8)
    DC = C // 128          # d chunks for output (2)

    with tc.tile_pool(name="persist", bufs=1) as pp:
        # ---- constants (from DRAM inputs) ----
        wt3_sb = pp.tile([128, CC, C], F16, tag="wt3")
        bcol_sb = pp.tile([128, DC], FP32, tag="bcol")
        esel_sb = pp.tile([128, T * T], F16, tag="esel")
        i16_sb = pp.tile([96, T], FP32, tag="i16")
        diag_sb = pp.tile([T, T], FP32, tag="diag")
        tcol_sb = pp.tile([T, 1], FP32, tag="tcol")
        # esel is needed immediately by the stats matmuls; the rest later
        nc.sync.dma_start(esel_sb[:], handles["esel"].ap())
        nc.sync.dma_start(i16_sb[:], handles["i16"].ap())

        # ---- persistent state ----
        f16_sb = pp.tile([128, CC, T * P], F16, tag="f16")
        # stats rows: sm at partitions 0..15, ss at 32..47, gm at 64..79
        stats_sb = pp.tile([96, P], FP32, tag="stats")
        sm_sb = stats_sb[0:T, :]
        ss_sb = stats_sb[32:32 + T, :]
        gm_sb = stats_sb[64:64 + T, :]
        smt_sb = pp.tile([128, PB, T], FP32, tag="smT")
        rnt_sb = pp.tile([128, PB, T], FP32, tag="rnT")
        rst_sb = pp.tile([128, PB, T], FP32, tag="rsT")
        spt_sb = pp.tile([128, PB, T], FP32, tag="SpT")
        mpt_sb = pp.tile([128, PB, T], FP32, tag="MpT")
        scores_sb = pp.tile([T, T], FP32, tag="scores")
        maxv_sb = pp.tile([T, 8], FP32, tag="maxv")
        maxi_sb = pp.tile([T, 8], U32, tag="maxi")
        idxf_sb = pp.tile([T, K], FP32, tag="idxf")
        gt_sb = pp.tile([T, K], FP32, tag="gt")
        cidxf_sb = pp.tile([T, K], FP32, tag="cidxf")
        cidx_sb = pp.tile([T, K], I32, tag="cidx")
        row_sb = pp.tile([1, T * K], I32, tag="row")

        # ================= Phase A: stream in, stats =================
        with tc.tile_pool(name="statsps", bufs=1, space="PSUM") as sps, \
             tc.tile_pool(name="stream", bufs=4) as sp:
            # per (stat, p-half) psum banks; stat j writes partition strip 32j
            # so the 3 stats' matmuls col-tile and run concurrently on the PE
            st_ps = [[sps.tile([96, 512], FP32, tag=f"stp{ph}_{j}",
                               name=f"stp{ph}_{j}") for j in range(3)]
                     for ph in range(PH)]

            last_sq = None
            for t in range(T):
                fch = sp.tile([128, CC, P], FP32, tag="fch")
                for cc in range(CC):
                    nc.sync.dma_start(fch[:, cc, :],
                                      feat_h[t, cc * 128:(cc + 1) * 128, :])
                # fp16 copy: used by the stats matmuls AND the gather phase
                f16c = f16_sb[:, :, t * P:(t + 1) * P]
                nc.vector.tensor_copy(f16c, fch[:])
                sq = sp.tile([128, CC, P], F16, tag="sq")
                nc.scalar.activation(sq[:], fch[:], AF.Square)
                last_sq = sq
                gsc = sp.tile([128, CC, P], F16, tag="gsc")
                nc.vector.tensor_scalar(gsc[:], fch[:], 0.0, None, OP.is_gt)
                st = (t == 0)
                sx = (t == T - 1)
                lhs = esel_sb[:, T * t:T * (t + 1)]
                for cc in range(CC):
                    for ph in range(PH):
                        sl = slice(ph * 512, (ph + 1) * 512)
                        for j, src in enumerate((f16c, sq, gsc)):
                            nc.tensor.matmul(
                                st_ps[ph][j][32 * j:32 * j + T, :], lhs,
                                src[:, cc, sl],
                                start=st and cc == 0, stop=sx and cc == CC - 1,
                                tile_position=(0, 32 * j))

            # preload the Sqrt ACT table while the stats evacuate (depends on
            # the last Square so it cannot run early and evict its table)
            dummy_sb = sp.tile([1, 1], FP32, tag="dummy")
            nc.scalar.activation(dummy_sb[:], last_sq[0:1, 0, 0:1], AF.Sqrt)

            # ss evacuates through ACT with fused sqrt (-> rn); sm/gm copy out
            # on the DVE in parallel
            for ph in range(PH):
                sl = slice(ph * 512, (ph + 1) * 512)
                nc.scalar.activation(stats_sb[32:32 + T, sl],
                                     st_ps[ph][1][32:32 + T, :], AF.Sqrt)
            for ph in range(PH):
                sl = slice(ph * 512, (ph + 1) * 512)
                for j in (0, 2):
                    nc.vector.tensor_copy(
                        stats_sb[32 * j:32 * j + T, sl],
                        st_ps[ph][j][32 * j:32 * j + T, :])

        # remaining constants (needed from phase B onward)
        for name, t_ in (("wt3", wt3_sb), ("bcol", bcol_sb),
                         ("diagbig", diag_sb), ("tcol", tcol_sb)):
            nc.sync.dma_start(t_[:], handles[name].ap())

        # ================= Phase B: scores + top-k =================
        with tc.tile_pool(name="bps", bufs=2, space="PSUM") as bps:
            # transpose raw stats to p-major; the ss strip already holds
            # rn = sqrt(ss), so only recip + mul remain, on 128 partitions
            for src, ibase, dst in ((ss_sb, 32, rnt_sb), (sm_sb, 0, smt_sb),
                                    (gm_sb, 64, mpt_sb)):
                tr = bps.tile([128, PB * T], FP32, tag="tr", name="tr")
                ident = i16_sb[ibase:ibase + T, :]
                for pb in range(PB):
                    nc.tensor.transpose(tr[:, pb * T:(pb + 1) * T],
                                        src[:, pb * 128:(pb + 1) * 128], ident)
                nc.scalar.copy(dst[:, :, :], tr[:])
            nc.vector.reciprocal(rst_sb[:], rnt_sb[:])
            nc.vector.tensor_mul(spt_sb[:], smt_sb[:], rst_sb[:])

            sc_ps = bps.tile([T, T], FP32, tag="scps")
            for pb in range(PB):
                nc.tensor.matmul(sc_ps[:], spt_sb[:, pb, :], mpt_sb[:, pb, :],
                                 start=(pb == 0), stop=(pb == PB - 1))
            # exclude s == t, move to SBUF
            nc.vector.tensor_sub(scores_sb[:], sc_ps[:], diag_sb[:])

            nc.vector.max(maxv_sb[:], scores_sb[:])
            nc.vector.max_index(maxi_sb[:], maxv_sb[:], scores_sb[:])
            # compressed index c* = s* - (s* > t)   (faithful reference bug)
            nc.vector.tensor_copy(idxf_sb[:], maxi_sb[:, 0:K])
            nc.vector.tensor_scalar(gt_sb[:], idxf_sb[:], tcol_sb[:, 0:1], None, OP.is_gt)
            nc.vector.tensor_sub(cidxf_sb[:], idxf_sb[:], gt_sb[:])
            nc.vector.tensor_copy(cidx_sb[:], cidxf_sb[:])
            nc.sync.dma_start(row_sb[:], cidx_sb[:])
            # debug outputs
            nc.sync.dma_start(sdbg_h.ap(), scores_sb[:])
            nc.sync.dma_start(idbg_h.ap(), row_sb[:])

        # ================= Phase C: gather-combine + linear =================
        with tc.tile_pool(name="cps", bufs=4, space="PSUM") as cps, \
             tc.tile_pool(name="cpool", bufs=3) as cp:
            # batched register loads for all T*K gather offsets (<=32 per inst)
            avals = []
            half = T * K // 2
            for lo in range(0, T * K, half):
                _, v = nc.values_load_multi_w_load_instructions(
                    row_sb[0:1, lo:lo + half],
                    engines=bass.OrderedSet([mybir.EngineType.DVE]),
                    min_val=0, max_val=T - 2,
                    skip_runtime_bounds_check=True,
                )
                avals.extend(v)
            for t in range(T):
                vals = avals[K * t:K * (t + 1)]
                mf16 = cp.tile([128, CC, P], F16, tag="mf16")
                a0 = f16_sb[:, :, bass.ds(vals[0] * P, P)]
                a1 = f16_sb[:, :, bass.ds(vals[1] * P, P)]
                nc.vector.tensor_add(mf16[:], a0, a1)
                for k in range(2, K):
                    ak = f16_sb[:, :, bass.ds(vals[k] * P, P)]
                    nc.vector.tensor_add(mf16[:], mf16[:], ak)
                for dc in range(DC):
                    osb = cp.tile([128, P], FP32, tag="osb", bufs=4)
                    # [128,1024] psum tile spans 2 banks; each 512-half is its
                    # own accumulation group -> one wide IDENTITY evac per dc
                    po = cps.tile([128, P], FP32, tag="po")
                    for ph in range(PH):
                        for cc in range(CC):
                            nc.tensor.matmul(
                                po[:, ph * 512:(ph + 1) * 512],
                                wt3_sb[:, cc, dc * 128:(dc + 1) * 128],
                                mf16[:, cc, ph * 512:(ph + 1) * 512],
                                start=(cc == 0), stop=(cc == CC - 1),
                            )
                    nc.scalar.activation(osb[:], po[:],
                                         AF.Identity, bias=bcol_sb[:, dc:dc + 1])
                    nc.sync.dma_start(out_h[t, dc * 128:(dc + 1) * 128, :], osb[:])


def build_program(T=16, C=256, P=1024, K=3):
    nc = bacc.Bacc("TRN2", target_bir_lowering=False, debug=False,
                   num_devices=N_CORES)
    handles = {}
    handles["features"] = nc.dram_tensor("features", [T, C, P], FP32,
                                         kind="ExternalInput")
    for name, shape, dt in (
        ("wt3", [128, C // 128, C], F16),
        ("bcol", [128, C // 128], FP32),
        ("esel", [128, T * T], F16),
        ("i16", [96, T], FP32),
        ("diagbig", [T, T], FP32),
        ("tcol", [T, 1], FP32),
    ):
        handles[name] = nc.dram_tensor(name, shape, dt, kind="ExternalInput")
    handles["out"] = nc.dram_tensor("out", [T, C, P], FP32, kind="ExternalOutput")
    handles["scores_dbg"] = nc.dram_tensor("scores_dbg", [T, T], FP32,
                                           kind="ExternalOutput")
    handles["idx_dbg"] = nc.dram_tensor("idx_dbg", [1, T * K], I32,
                                        kind="ExternalOutput")

    with tile.TileContext(nc) as tc:
        _emit(nc, tc, T, C, P, K, handles)
    nc.compile()
    return nc


def _host_consts(W, b, T, C, K):
    consts = {}
    wt3 = (np.asarray(W, np.float32).T / float(K)).astype(np.float32)  # [C, C] (c, d)
    # [c_in(partition), cc, d] in fp16
    w4 = wt3.reshape(C // 128, 128, C).transpose(1, 0, 2)
    consts["wt3"] = np.ascontiguousarray(w4.astype(np.float16))
    consts["bcol"] = np.ascontiguousarray(
        np.asarray(b, np.float32).reshape(C // 128, 128).T)
    esel = np.zeros((128, T * T), np.float16)
    for t in range(T):
        esel[:, T * t + t] = 1.0
    consts["esel"] = esel
    i16 = np.zeros((96, T), np.float32)
    for r in (0, 32, 64):
        i16[r:r + T, :] = np.eye(T, dtype=np.float32)
    consts["i16"] = i16
    consts["diagbig"] = (np.eye(T, dtype=np.float32) * BIG).astype(np.float32)
    consts["tcol"] = np.arange(T, dtype=np.float32).reshape(T, 1)
    return consts


_CACHE = {}


def kernel(features, W, b, top_k):
    features = np.asarray(features, np.float32)
    T, B, C, H, Wd = features.shape
    P = H * Wd
    K = int(top_k)
    assert B == N_CORES and C == 256 and P == 1024 and T == 16 and K == 3

    key = (T, C, P, K)
    if key not in _CACHE:
        _CACHE[key] = build_program(T, C, P, K)
    nc = _CACHE[key]

    consts = _host_consts(W, b, T, C, K)
    feat = features.reshape(T, B, C, P)
    in_maps = [
        {"features": np.ascontiguousarray(feat[:, i]), **consts}
        for i in range(N_CORES)
    ]
    res = run_bass_kernel_spmd(nc, in_maps, list(range(N_CORES)))
    out = np.stack([res.results[i]["out"] for i in range(N_CORES)], axis=1)
    return np.ascontiguousarray(out.reshape(T, B, C, H, Wd))
